# revision 17
# baseline (speedup 1.0000x reference)
"""Trainium2 Bass kernel for the CRA relation module.

Math: the reference computes, per sample,
    phi_x = relu((x@W1+b1)*g1+be1), phi_y likewise,  cat_phi = [phi_x; phi_y]
    A = cat_phi cat_phi^T (symmetric!),  R = [A | A^T] = [A | A]
    W = (cat_phi@W3+b3)@W5a + (R@W4+b4)@W5b + b5
    out = x * W[:196] + y * W[196:]
Because A is symmetric and everything after A is linear into a scalar per
token, the relation pipeline collapses to per-sample matvecs:
    u3 = W3@W5a, u4 = W4@W5b, z = u4[:392]+u4[392:], c0 = b3@W5a+b4@W5b+b5
    s  = u3 + phi_x^T z[:196] + phi_y^T z[196:]          (768-vector)
    out = x*(phi_x@s + c0) + y*(phi_y@s + c0)

Device design:
  - dense 1x1 convs in fp8e4m3 DoubleRow (2 contraction rows/cycle),
    feature-major (cin on partitions), 2 samples per psum pass (N=392)
  - relu eviction on ACT -> fp16 phi tiles
  - s-reduction: scalar_tensor_tensor multiply-accumulate, split DVE/GPSIMD
  - per-token scalar w via PE broadcast matvec (replicated across partitions)
  - finish out = x*wx + y*wy as per-d 2D fp16 DVE multiplies (2x packed
    mode) with the combine add on GPSIMD in bf16 (its fast dtype).
"""

import ml_dtypes
import numpy as np
from contextlib import ExitStack

import concourse.bass as bass
import concourse.tile as tile
import concourse.mybir as mybir
from concourse.bass_utils import run_bass_kernel_spmd

F32 = mybir.dt.float32
F16 = mybir.dt.float16
BF16 = mybir.dt.bfloat16
FP8 = mybir.dt.float8e4
ALU = mybir.AluOpType
ACTF = mybir.ActivationFunctionType
F16_NP = np.float16
FP8_NP = ml_dtypes.float8_e4m3

B, N, C = 128, 196, 768
NCORES = 8
S = B // NCORES          # 16 samples per core
G = 2                    # samples per weight pass (moving N = 392 <= 512 fp32)
NG = S // G              # 8 groups (= token pairs) per core
DT = C // 128            # 6 feature tiles
KP = DT // 2             # 3 contraction pair-tiles (fp8 DoubleRow)
W2T = 2 * N              # 392
W2TP = 400               # fp8 token block padded so pair-stride % 16 == 0


def build_bass(c0: float) -> bass.Bass:
    nc = bass.Bass()
    # GEMM copies: [group, part, kpair, j, token(padded)] fp8
    xm_d = nc.declare_dram_parameter("xm", [NG, 128, KP, 2, W2TP], FP8, isOutput=False)
    ym_d = nc.declare_dram_parameter("ym", [NG, 128, KP, 2, W2TP], FP8, isOutput=False)
    w1_d = nc.declare_dram_parameter("w1", [DT, 128, KP, 2, 128], FP8, isOutput=False)
    w2_d = nc.declare_dram_parameter("w2", [DT, 128, KP, 2, 128], FP8, isOutput=False)
    # finish copies feature-major fp16
    xf_d = nc.declare_dram_parameter("xf", [NG, 128, DT * W2T], F16, isOutput=False)
    yf_d = nc.declare_dram_parameter("yf", [NG, 128, DT * W2T], F16, isOutput=False)
    zb_d = nc.declare_dram_parameter("zb", [128, W2T], F16, isOutput=False)
    u3_d = nc.declare_dram_parameter("u3", [128, DT], F32, isOutput=False)
    b1_d = nc.declare_dram_parameter("b1", [128, DT], F32, isOutput=False)
    b2_d = nc.declare_dram_parameter("b2", [128, DT], F32, isOutput=False)
    out_d = nc.declare_dram_parameter("out", [S, 128, DT * N], F16, isOutput=True)

    with tile.TileContext(nc) as tc, ExitStack() as ctx:
        const = ctx.enter_context(tc.tile_pool(name="const", bufs=1))
        xin = ctx.enter_context(tc.tile_pool(name="xin", bufs=3))
        fin = ctx.enter_context(tc.tile_pool(name="fin", bufs=3))
        phip = ctx.enter_context(tc.tile_pool(name="phi", bufs=3))
        sp = ctx.enter_context(tc.tile_pool(name="sp", bufs=3))
        gp = ctx.enter_context(tc.tile_pool(name="gp", bufs=2))
        wp = ctx.enter_context(tc.tile_pool(name="wp", bufs=2))
        op = ctx.enter_context(tc.tile_pool(name="op", bufs=2))
        ps = ctx.enter_context(tc.tile_pool(name="ps", bufs=2, space="PSUM"))

        def dma_gemm_group(g, eng=nc.sync, split=False):
            xm = xin.tile([128, KP * 2 * W2TP], FP8, tag="xm", name="xm")
            ym = xin.tile([128, KP * 2 * W2TP], FP8, tag="ym", name="ym")
            if split:
                blk = 2 * W2TP
                engs = [nc.scalar, nc.gpsimd, nc.scalar,
                        nc.gpsimd, nc.scalar, nc.gpsimd]
                for k in range(KP):
                    engs[2 * k].dma_start(out=xm[:, k * blk:(k + 1) * blk],
                                          in_=xm_d[g][:, k])
                    engs[2 * k + 1].dma_start(out=ym[:, k * blk:(k + 1) * blk],
                                              in_=ym_d[g][:, k])
            else:
                eng.dma_start(out=xm[:], in_=xm_d[g])
                eng.dma_start(out=ym[:], in_=ym_d[g])
            return xm, ym

        def dma_fin_group(g):
            xf = fin.tile([128, DT * W2T], F16, tag="xf", name="xf")
            yf = fin.tile([128, DT * W2T], F16, tag="yf", name="yf")
            nc.sync.dma_start(out=xf[:], in_=xf_d[g])
            nc.sync.dma_start(out=yf[:], in_=yf_d[g])
            return xf, yf

        # First-needed bytes first. Issue the critical prefix from several
        # engines: the issuing sequencer serializes dma_starts (~0.25us
        # each), so a single engine would gate the pipeline start.
        w1_sb, w2_sb = [], []
        for d in range(DT):
            t1 = const.tile([128, KP * 2 * 128], FP8, tag=f"w1_{d}")
            w1_sb.append(t1)
            t2 = const.tile([128, KP * 2 * 128], FP8, tag=f"w2_{d}")
            w2_sb.append(t2)
        wblk = 2 * 128
        for k in range(KP):
            nc.scalar.dma_start(out=w1_sb[0][:, k * wblk:(k + 1) * wblk],
                                in_=w1_d[0][:, k])
            nc.gpsimd.dma_start(out=w2_sb[0][:, k * wblk:(k + 1) * wblk],
                                in_=w2_d[0][:, k])
        zb = const.tile([128, W2T], F16, tag="zb")
        nc.gpsimd.dma_start(out=zb[:], in_=zb_d[:, :])
        u3 = const.tile([128, DT], F32, tag="u3")
        nc.gpsimd.dma_start(out=u3[:], in_=u3_d[:, :])
        b1t = const.tile([128, DT], F32, tag="b1")
        nc.scalar.dma_start(out=b1t[:], in_=b1_d[:, :])
        b2t = const.tile([128, DT], F32, tag="b2")
        nc.scalar.dma_start(out=b2t[:], in_=b2_d[:, :])
        xy0 = dma_gemm_group(0, split=True)
        nc.scalar.dma_start(out=w1_sb[1][:], in_=w1_d[1])
        nc.gpsimd.dma_start(out=w2_sb[1][:], in_=w2_d[1])
        for d in range(2, DT):
            nc.sync.dma_start(out=w1_sb[d][:], in_=w1_d[d])
            nc.sync.dma_start(out=w2_sb[d][:], in_=w2_d[d])
        # Absorb the bias-tile DMA deps into ACT program order now, so the
        # relu evictions later only ever wait on the PE semaphore.
        warm1 = const.tile([128, 1], F32, tag="warm1")
        warm2 = const.tile([128, 1], F32, tag="warm2")
        nc.scalar.activation(warm1[:], b1t[:, 0:1], ACTF.Copy)
        nc.scalar.activation(warm2[:], b2t[:, 0:1], ACTF.Copy)

        def emit_mm(psum, w_sb, xg):
            wv = w_sb[:].rearrange("p (k j m) -> p k j m", k=KP, j=2)
            xv = xg[:].rearrange("p (k j t) -> p k j t", k=KP, j=2)
            for k in range(KP):
                nc.tensor.matmul(
                    psum[:], wv[:, k], xv[:, k, :, 0:W2T],
                    start=(k == 0), stop=(k == KP - 1),
                    perf_mode=mybir.MatmulPerfMode.DoubleRow)

        def emit_mains(g, xy):
            xg, yg = xy
            # One phi tile PER d-block so consumers' dependencies are exact.
            # Layout per d: [a: x(196)|y(196) | b: x(196)|y(196)].
            phd = [phip.tile([128, G * W2T], F16, tag=f"phd_{d}",
                             name=f"phd_{d}") for d in range(DT)]
            for d in range(DT):
                psx = ps.tile([128, W2T], F32, tag="psx", name="psx", bufs=3)
                psy = ps.tile([128, W2T], F32, tag="psy", name="psy", bufs=3)
                emit_mm(psx, w1_sb[d], xg)
                emit_mm(psy, w2_sb[d], yg)
                phv = phd[d][:].rearrange("p (i s t) -> p i s t", i=G, s=2)
                nc.scalar.activation(phv[:, :, 0, :],
                                     psx[:].rearrange("p (i t) -> p i t", i=G),
                                     ACTF.Relu, bias=b1t[:, d:d + 1])
                nc.scalar.activation(phv[:, :, 1, :],
                                     psy[:].rearrange("p (i t) -> p i t", i=G),
                                     ACTF.Relu, bias=b2t[:, d:d + 1])
            return phd

        def emit_head(g, phd):
            # s = u3 + phi^T z via fused multiply-accumulate; the product is
            # garbage (only accum matters) and goes to a stride-0 sbuf junk
            # tile, keeping DVE off the slow PSUM write path. Then
            # w = phi @ s + c0 on PE via a stride-0 broadcast lhsT.
            wxys = []
            for i in range(G):
                t_sb = sp.tile([128, DT], F32, tag=f"t_{i}", name=f"t_{i}")
                s_sb = sp.tile([128, DT], F16, tag=f"s_{i}", name=f"s_{i}")
                junk = gp.tile([128, W2T], F16, tag="junk", name="junk", bufs=3)
                # The product is garbage (only accum matters); a dense 2-byte
                # sbuf out keeps DVE off the PSUM path (and off the broadcast
                # write-conflict path), eligible for the 2x packed mode.
                for d in range(DT):
                    nc.vector.scalar_tensor_tensor(
                        out=junk[:],
                        in0=phd[d][:, i * W2T:(i + 1) * W2T],
                        scalar=1.0, in1=zb[:],
                        op0=ALU.mult, op1=ALU.mult,
                        accum_out=t_sb[:, d:d + 1])
                nc.gpsimd.tensor_tensor(s_sb[:], t_sb[:], u3[:], ALU.add)
                psw = ps.tile([128, W2T], F32, tag="psw", name="psw", bufs=2)
                for d in range(DT):
                    nc.tensor.matmul(
                        psw[:], s_sb[:, d:d + 1].broadcast_to([128, 128]),
                        phd[d][:, i * W2T:(i + 1) * W2T],
                        start=(d == 0), stop=(d == DT - 1))
                wxy = sp.tile([128, W2T], F16, tag=f"wxy_{i}", name=f"wxy_{i}")
                nc.scalar.activation(wxy[:], psw[:], ACTF.Copy, bias=c0)
                wxys.append(wxy)
            return wxys

        def emit_finish(g, xf, yf, wxys, drain=False):
            for i in range(G):
                wxy = wxys[i]
                osb = op.tile([128, DT * N], F16, tag=f"osb_{i}", name=f"osb_{i}")
                tmp = op.tile([128, DT * N], F16, tag=f"tmp_{i}", name=f"tmp_{i}")
                for d in range(DT):
                    xv = xf[:, d * W2T + i * N: d * W2T + (i + 1) * N]
                    yv = yf[:, d * W2T + i * N: d * W2T + (i + 1) * N]
                    nc.vector.tensor_tensor(
                        tmp[:, d * N:(d + 1) * N], wxy[:, N:W2T], yv, ALU.mult)
                    nc.vector.tensor_tensor(
                        osb[:, d * N:(d + 1) * N], wxy[:, 0:N], xv, ALU.mult)
                    if drain:
                        # drain mode: combine + ship per-d on DVE so the out
                        # DMA streams while later d blocks still multiply
                        # (the serial gpsimd add + one fat DMA was the tail)
                        nc.vector.tensor_tensor(
                            osb[:, d * N:(d + 1) * N], osb[:, d * N:(d + 1) * N],
                            tmp[:, d * N:(d + 1) * N], ALU.add)
                        (nc.scalar, nc.gpsimd, nc.sync)[d % 3].dma_start(
                            out=out_d[G * g + i][:, d * N:(d + 1) * N],
                            in_=osb[:, d * N:(d + 1) * N])
                if not drain:
                    nc.gpsimd.tensor_tensor(osb[:], osb[:], tmp[:], ALU.add)
                    # 3 chunks on 3 queues: one 300KB dma_start pins a single
                    # ~22GB/s queue for ~14us and was setting the kernel end
                    ch = 2 * N
                    for ci, eng in enumerate((nc.scalar, nc.gpsimd, nc.sync)):
                        eng.dma_start(
                            out=out_d[G * g + i][:, ci * ch:(ci + 1) * ch],
                            in_=osb[:, ci * ch:(ci + 1) * ch])

        # 3-stage software pipeline: PE runs group g's dense matmuls while
        # group g-1's reduction chain feeds its matvec and group g-2's
        # finish drains.
        mains, heads, fins = {}, {}, {}
        xy = xy0
        for g in range(NG):
            mains[g] = emit_mains(g, xy)
            if g + 1 < NG:
                xy = dma_gemm_group(g + 1)
            fins[g] = dma_fin_group(g)
            if g >= 1:
                heads[g - 1] = emit_head(g - 1, mains[g - 1])
            if 2 <= g:
                emit_finish(g - 2, *fins[g - 2], heads[g - 2])
        heads[NG - 1] = emit_head(NG - 1, mains[NG - 1])
        emit_finish(NG - 2, *fins[NG - 2], heads[NG - 2], drain=True)
        emit_finish(NG - 1, *fins[NG - 1], heads[NG - 1], drain=True)

    _split_multi_waits(nc)
    return nc


def _split_multi_waits(nc):
    """This walrus build accepts at most ONE sync-wait command per TPB
    instruction; the Tile scheduler happily emits several. Hoist all but the
    last wait of each instruction onto same-engine EventSemaphore ops placed
    immediately before it (engine program order is the within-block
    subsequence, so this preserves semantics)."""
    import json
    data = json.loads(nc.to_json_bytes())
    n = 0
    for fn in data["functions"]:
        for blk in fn["blocks"]:
            out = []
            for inst in blk["instructions"]:
                si = inst.get("sync_info")
                ow = (si or {}).get("on_wait") or []
                if len(ow) > 1:
                    for w in ow[:-1]:
                        n += 1
                        out.append({
                            "name": f"eswait_{n}",
                            "opcode": "EventSemaphore",
                            "engine": inst["engine"],
                            "ins": [],
                            "outs": [],
                            "sync_info": {"on_wait": [w], "on_update": []},
                        })
                    si["on_wait"] = [ow[-1]]
                out.append(inst)
            blk["instructions"] = out
    nc.m = mybir.module_from_json_bytes(json.dumps(data).encode())
    return nc


def prep_host(inputs: dict):
    x = np.ascontiguousarray(np.asarray(inputs["x"], dtype=np.float32))
    y = np.ascontiguousarray(np.asarray(inputs["y"], dtype=np.float32))
    W1 = np.asarray(inputs["W1"], dtype=np.float32)
    W2 = np.asarray(inputs["W2"], dtype=np.float32)
    g1 = np.asarray(inputs["g1"], dtype=np.float32)
    g2 = np.asarray(inputs["g2"], dtype=np.float32)
    b1 = np.asarray(inputs["b1"], dtype=np.float32)
    b2 = np.asarray(inputs["b2"], dtype=np.float32)
    be1 = np.asarray(inputs["be1"], dtype=np.float32)
    be2 = np.asarray(inputs["be2"], dtype=np.float32)
    W3 = np.asarray(inputs["W3"], dtype=np.float32)
    b3 = np.asarray(inputs["b3"], dtype=np.float32)
    W4 = np.asarray(inputs["W4"], dtype=np.float32)
    b4 = np.asarray(inputs["b4"], dtype=np.float32)
    W5 = np.asarray(inputs["W5"], dtype=np.float32)
    b5 = np.asarray(inputs["b5"], dtype=np.float32)

    W1p = W1 * g1[None, :]
    W2p = W2 * g2[None, :]
    b1p = b1 * g1 + be1
    b2p = b2 * g2 + be2
    W5a, W5b = W5[:C, 0], W5[C:, 0]
    u3 = (W3 @ W5a).astype(np.float32)
    u4 = (W4 @ W5b).astype(np.float32)
    z = (u4[:2 * N] + u4[2 * N:]).astype(np.float32)
    c0 = float(b3 @ W5a + b4 @ W5b + b5[0])

    def pack_w(w):
        # [C, C] -> [DT, 128, KP, 2, 128]: [m-block d, part p, kpair, j, m]
        # = w[(2*kpair+j)*128+p, d*128+m]
        v = w.astype(FP8_NP).reshape(KP, 2, 128, DT, 128)
        return np.ascontiguousarray(v.transpose(3, 2, 0, 1, 4))

    def pack_gemm(a):
        # [B,N,C] -> [M, NG, 128, KP, 2, 400]: token blocks [a(196)|b(196)|pad]
        at = a.transpose(0, 2, 1).reshape(NCORES, S, KP, 2, 128, N)
        pair = at.reshape(NCORES, NG, G, KP, 2, 128, N)
        gg = np.concatenate([pair[:, :, 0], pair[:, :, 1]], axis=-1)
        gg = gg.transpose(0, 1, 4, 2, 3, 5)  # [M,NG,128,KP,2,392]
        out = np.zeros((NCORES, NG, 128, KP, 2, W2TP), dtype=FP8_NP)
        out[..., :W2T] = gg.astype(FP8_NP)
        return out

    def pack_fin(a):
        at = a.transpose(0, 2, 1).reshape(NCORES, S, DT, 128, N)
        pair = at.reshape(NCORES, NG, G, DT, 128, N)
        gg = np.concatenate([pair[:, :, 0], pair[:, :, 1]], axis=-1)
        return np.ascontiguousarray(
            gg.transpose(0, 1, 3, 2, 4).reshape(NCORES, NG, 128, DT * W2T)
            .astype(F16_NP))

    XM, YM = pack_gemm(x), pack_gemm(y)
    XF, YF = pack_fin(x), pack_fin(y)
    zb = np.ascontiguousarray(np.broadcast_to(z[None, :], (128, W2T)).astype(F16_NP))
    u3t = np.ascontiguousarray(u3.reshape(DT, 128).T)
    b1t = np.ascontiguousarray(b1p.reshape(DT, 128).T)
    b2t = np.ascontiguousarray(b2p.reshape(DT, 128).T)

    in_maps = []
    for cidx in range(NCORES):
        in_maps.append({
            "xm": XM[cidx], "ym": YM[cidx], "xf": XF[cidx], "yf": YF[cidx],
            "w1": pack_w(W1p), "w2": pack_w(W2p),
            "zb": zb, "u3": u3t, "b1": b1t, "b2": b2t,
        })
    return in_maps, c0, x, y


def unpack_out(results) -> np.ndarray:
    outs = []
    for cidx in range(NCORES):
        o = np.asarray(results[cidx]["out"]).astype(np.float32)  # [S, 128, DT*N]
        o = o.reshape(S, 128, DT, N).transpose(0, 2, 1, 3).reshape(S, C, N)
        outs.append(o.transpose(0, 2, 1))     # [S, N, C]
    return np.ascontiguousarray(np.concatenate(outs, axis=0))


def kernel(**inputs) -> np.ndarray:
    in_maps, c0, _, _ = prep_host(inputs)
    nc = build_bass(c0)
    res = run_bass_kernel_spmd(nc, in_maps, list(range(NCORES)))
    return unpack_out(res.results)


# revision 18
# speedup vs baseline: 1.1958x; 1.1958x over previous
"""Trainium2 Bass kernel for the CRA relation module.

Math: the reference computes, per sample,
    phi_x = relu((x@W1+b1)*g1+be1), phi_y likewise,  cat_phi = [phi_x; phi_y]
    A = cat_phi cat_phi^T (symmetric!),  R = [A | A^T] = [A | A]
    W = (cat_phi@W3+b3)@W5a + (R@W4+b4)@W5b + b5
    out = x * W[:196] + y * W[196:]
Because A is symmetric and everything after A is linear into a scalar per
token, the relation pipeline collapses to per-sample matvecs:
    u3 = W3@W5a, u4 = W4@W5b, z = u4[:392]+u4[392:], c0 = b3@W5a+b4@W5b+b5
    s  = u3 + phi_x^T z[:196] + phi_y^T z[196:]          (768-vector)
    out = x*(phi_x@s + c0) + y*(phi_y@s + c0)

Device design:
  - dense 1x1 convs in fp8e4m3 DoubleRow (2 contraction rows/cycle),
    feature-major (cin on partitions), 2 samples per psum pass (N=392)
  - relu eviction on ACT -> fp16 phi tiles
  - s-reduction: scalar_tensor_tensor multiply-accumulate, split DVE/GPSIMD
  - per-token scalar w via PE broadcast matvec (replicated across partitions)
  - finish out = x*wx + y*wy as per-d 2D fp16 DVE multiplies (2x packed
    mode) with the combine add on GPSIMD in bf16 (its fast dtype).
"""

import ml_dtypes
import numpy as np
from contextlib import ExitStack

import concourse.bass as bass
import concourse.tile as tile
import concourse.mybir as mybir
from concourse.bass_utils import run_bass_kernel_spmd

F32 = mybir.dt.float32
F16 = mybir.dt.float16
BF16 = mybir.dt.bfloat16
FP8 = mybir.dt.float8e4
ALU = mybir.AluOpType
ACTF = mybir.ActivationFunctionType
F16_NP = np.float16
FP8_NP = ml_dtypes.float8_e4m3

B, N, C = 128, 196, 768
NCORES = 8
S = B // NCORES          # 16 samples per core
G = 2                    # samples per weight pass (moving N = 392 <= 512 fp32)
NG = S // G              # 8 groups (= token pairs) per core
DT = C // 128            # 6 feature tiles
KP = DT // 2             # 3 contraction pair-tiles (fp8 DoubleRow)
W2T = 2 * N              # 392
W2TP = 400               # fp8 token block padded so pair-stride % 16 == 0


def build_bass(c0: float) -> bass.Bass:
    nc = bass.Bass()
    # GEMM copies: [group, part, kpair, j, token(padded)] fp8
    xm_d = nc.declare_dram_parameter("xm", [NG, 128, KP, 2, W2TP], FP8, isOutput=False)
    ym_d = nc.declare_dram_parameter("ym", [NG, 128, KP, 2, W2TP], FP8, isOutput=False)
    w1_d = nc.declare_dram_parameter("w1", [DT, 128, KP, 2, 128], FP8, isOutput=False)
    w2_d = nc.declare_dram_parameter("w2", [DT, 128, KP, 2, 128], FP8, isOutput=False)
    # finish copies feature-major fp16
    xf_d = nc.declare_dram_parameter("xf", [NG, 128, DT * W2T], F16, isOutput=False)
    yf_d = nc.declare_dram_parameter("yf", [NG, 128, DT * W2T], F16, isOutput=False)
    zb_d = nc.declare_dram_parameter("zb", [128, W2T], F16, isOutput=False)
    u3_d = nc.declare_dram_parameter("u3", [128, DT], F32, isOutput=False)
    b1_d = nc.declare_dram_parameter("b1", [128, DT], F32, isOutput=False)
    b2_d = nc.declare_dram_parameter("b2", [128, DT], F32, isOutput=False)
    out_d = nc.declare_dram_parameter("out", [S, 128, DT * N], F16, isOutput=True)

    with tile.TileContext(nc) as tc, ExitStack() as ctx:
        const = ctx.enter_context(tc.tile_pool(name="const", bufs=1))
        xin = ctx.enter_context(tc.tile_pool(name="xin", bufs=3))
        fin = ctx.enter_context(tc.tile_pool(name="fin", bufs=3))
        phip = ctx.enter_context(tc.tile_pool(name="phi", bufs=3))
        sp = ctx.enter_context(tc.tile_pool(name="sp", bufs=3))
        gp = ctx.enter_context(tc.tile_pool(name="gp", bufs=2))
        wp = ctx.enter_context(tc.tile_pool(name="wp", bufs=2))
        op = ctx.enter_context(tc.tile_pool(name="op", bufs=2))
        ps = ctx.enter_context(tc.tile_pool(name="ps", bufs=2, space="PSUM"))

        def dma_gemm_group(g, eng=nc.sync, split=False):
            xm = xin.tile([128, KP * 2 * W2TP], FP8, tag="xm", name="xm")
            ym = xin.tile([128, KP * 2 * W2TP], FP8, tag="ym", name="ym")
            if split:
                blk = 2 * W2TP
                engs = [nc.scalar, nc.gpsimd, nc.sync, nc.scalar, nc.gpsimd, nc.sync]
                for k in range(KP):
                    engs[2 * k].dma_start(out=xm[:, k * blk:(k + 1) * blk],
                                          in_=xm_d[g][:, k])
                    engs[2 * k + 1].dma_start(out=ym[:, k * blk:(k + 1) * blk],
                                              in_=ym_d[g][:, k])
            else:
                eng.dma_start(out=xm[:], in_=xm_d[g])
                eng.dma_start(out=ym[:], in_=ym_d[g])
            return xm, ym

        def dma_fin_group(g):
            xf = fin.tile([128, DT * W2T], F16, tag="xf", name="xf")
            yf = fin.tile([128, DT * W2T], F16, tag="yf", name="yf")
            nc.sync.dma_start(out=xf[:], in_=xf_d[g])
            nc.sync.dma_start(out=yf[:], in_=yf_d[g])
            return xf, yf

        # First-needed bytes first. Issue the critical prefix from several
        # engines: the issuing sequencer serializes dma_starts (~0.25us
        # each), so a single engine would gate the pipeline start.
        w1_sb, w2_sb = [], []
        for d in range(DT):
            t1 = const.tile([128, KP * 2 * 128], FP8, tag=f"w1_{d}")
            w1_sb.append(t1)
            t2 = const.tile([128, KP * 2 * 128], FP8, tag=f"w2_{d}")
            w2_sb.append(t2)
        wblk = 2 * 128
        for k in range(KP):
            nc.scalar.dma_start(out=w1_sb[0][:, k * wblk:(k + 1) * wblk],
                                in_=w1_d[0][:, k])
            nc.gpsimd.dma_start(out=w2_sb[0][:, k * wblk:(k + 1) * wblk],
                                in_=w2_d[0][:, k])
        zb = const.tile([128, W2T], F16, tag="zb")
        nc.gpsimd.dma_start(out=zb[:], in_=zb_d[:, :])
        u3 = const.tile([128, DT], F32, tag="u3")
        nc.gpsimd.dma_start(out=u3[:], in_=u3_d[:, :])
        b1t = const.tile([128, DT], F32, tag="b1")
        nc.scalar.dma_start(out=b1t[:], in_=b1_d[:, :])
        b2t = const.tile([128, DT], F32, tag="b2")
        nc.scalar.dma_start(out=b2t[:], in_=b2_d[:, :])
        xy0 = dma_gemm_group(0, split=True)
        for d in range(1, DT):
            nc.sync.dma_start(out=w1_sb[d][:], in_=w1_d[d])
            nc.sync.dma_start(out=w2_sb[d][:], in_=w2_d[d])
        # Absorb the bias-tile DMA deps into ACT program order now, so the
        # relu evictions later only ever wait on the PE semaphore.
        warm1 = const.tile([128, 1], F32, tag="warm1")
        warm2 = const.tile([128, 1], F32, tag="warm2")
        nc.scalar.activation(warm1[:], b1t[:, 0:1], ACTF.Copy)
        nc.scalar.activation(warm2[:], b2t[:, 0:1], ACTF.Copy)

        def emit_mm(psum, w_sb, xg):
            wv = w_sb[:].rearrange("p (k j m) -> p k j m", k=KP, j=2)
            xv = xg[:].rearrange("p (k j t) -> p k j t", k=KP, j=2)
            for k in range(KP):
                nc.tensor.matmul(
                    psum[:], wv[:, k], xv[:, k, :, 0:W2T],
                    start=(k == 0), stop=(k == KP - 1),
                    perf_mode=mybir.MatmulPerfMode.DoubleRow)

        def emit_mains(g, xy):
            xg, yg = xy
            # One phi tile PER d-block so consumers' dependencies are exact.
            # Layout per d: [a: x(196)|y(196) | b: x(196)|y(196)].
            phd = [phip.tile([128, G * W2T], F16, tag=f"phd_{d}",
                             name=f"phd_{d}") for d in range(DT)]
            for d in range(DT):
                psx = ps.tile([128, W2T], F32, tag="psx", name="psx", bufs=3)
                psy = ps.tile([128, W2T], F32, tag="psy", name="psy", bufs=3)
                emit_mm(psx, w1_sb[d], xg)
                emit_mm(psy, w2_sb[d], yg)
                phv = phd[d][:].rearrange("p (i s t) -> p i s t", i=G, s=2)
                nc.scalar.activation(phv[:, :, 0, :],
                                     psx[:].rearrange("p (i t) -> p i t", i=G),
                                     ACTF.Relu, bias=b1t[:, d:d + 1])
                nc.scalar.activation(phv[:, :, 1, :],
                                     psy[:].rearrange("p (i t) -> p i t", i=G),
                                     ACTF.Relu, bias=b2t[:, d:d + 1])
            return phd

        def emit_head(g, phd):
            # s = u3 + phi^T z via fused multiply-accumulate; the product is
            # garbage (only accum matters) and goes to a stride-0 sbuf junk
            # tile, keeping DVE off the slow PSUM write path. Then
            # w = phi @ s + c0 on PE via a stride-0 broadcast lhsT.
            wxys = []
            for i in range(G):
                t_sb = sp.tile([128, DT], F32, tag=f"t_{i}", name=f"t_{i}")
                s_sb = sp.tile([128, DT], F16, tag=f"s_{i}", name=f"s_{i}")
                junk = gp.tile([128, W2T], F16, tag="junk", name="junk", bufs=3)
                # The product is garbage (only accum matters); a dense 2-byte
                # sbuf out keeps DVE off the PSUM path (and off the broadcast
                # write-conflict path), eligible for the 2x packed mode.
                for d in range(DT):
                    nc.vector.scalar_tensor_tensor(
                        out=junk[:],
                        in0=phd[d][:, i * W2T:(i + 1) * W2T],
                        scalar=1.0, in1=zb[:],
                        op0=ALU.mult, op1=ALU.mult,
                        accum_out=t_sb[:, d:d + 1])
                nc.gpsimd.tensor_tensor(s_sb[:], t_sb[:], u3[:], ALU.add)
                psw = ps.tile([128, W2T], F32, tag="psw", name="psw", bufs=2)
                for d in range(DT):
                    nc.tensor.matmul(
                        psw[:], s_sb[:, d:d + 1].broadcast_to([128, 128]),
                        phd[d][:, i * W2T:(i + 1) * W2T],
                        start=(d == 0), stop=(d == DT - 1))
                wxy = sp.tile([128, W2T], F16, tag=f"wxy_{i}", name=f"wxy_{i}")
                nc.scalar.activation(wxy[:], psw[:], ACTF.Copy, bias=c0)
                wxys.append(wxy)
            return wxys

        def emit_finish(g, xf, yf, wxys, drain=False):
            for i in range(G):
                wxy = wxys[i]
                osb = op.tile([128, DT * N], F16, tag=f"osb_{i}", name=f"osb_{i}")
                tmp = op.tile([128, DT * N], F16, tag=f"tmp_{i}", name=f"tmp_{i}")
                for d in range(DT):
                    xv = xf[:, d * W2T + i * N: d * W2T + (i + 1) * N]
                    yv = yf[:, d * W2T + i * N: d * W2T + (i + 1) * N]
                    nc.vector.tensor_tensor(
                        tmp[:, d * N:(d + 1) * N], wxy[:, N:W2T], yv, ALU.mult)
                    nc.vector.tensor_tensor(
                        osb[:, d * N:(d + 1) * N], wxy[:, 0:N], xv, ALU.mult)
                    if drain:
                        # drain mode: combine + ship per-d on DVE so the out
                        # DMA streams while later d blocks still multiply
                        # (the serial gpsimd add + one fat DMA was the tail)
                        nc.vector.tensor_tensor(
                            osb[:, d * N:(d + 1) * N], osb[:, d * N:(d + 1) * N],
                            tmp[:, d * N:(d + 1) * N], ALU.add)
                        nc.sync.dma_start(
                            out=out_d[G * g + i][:, d * N:(d + 1) * N],
                            in_=osb[:, d * N:(d + 1) * N])
                if not drain:
                    nc.gpsimd.tensor_tensor(osb[:], osb[:], tmp[:], ALU.add)
                    nc.sync.dma_start(out=out_d[G * g + i], in_=osb[:])

        # 3-stage software pipeline: PE runs group g's dense matmuls while
        # group g-1's reduction chain feeds its matvec and group g-2's
        # finish drains.
        mains, heads, fins = {}, {}, {}
        xy = xy0
        for g in range(NG):
            mains[g] = emit_mains(g, xy)
            if g + 1 < NG:
                xy = dma_gemm_group(g + 1)
            fins[g] = dma_fin_group(g)
            if g >= 1:
                heads[g - 1] = emit_head(g - 1, mains[g - 1])
            if 2 <= g:
                emit_finish(g - 2, *fins[g - 2], heads[g - 2])
        heads[NG - 1] = emit_head(NG - 1, mains[NG - 1])
        emit_finish(NG - 2, *fins[NG - 2], heads[NG - 2], drain=True)
        emit_finish(NG - 1, *fins[NG - 1], heads[NG - 1], drain=True)

    _split_multi_waits(nc)
    return nc


def _split_multi_waits(nc):
    """This walrus build accepts at most ONE sync-wait command per TPB
    instruction; the Tile scheduler happily emits several. Hoist all but the
    last wait of each instruction onto same-engine EventSemaphore ops placed
    immediately before it (engine program order is the within-block
    subsequence, so this preserves semantics)."""
    import json
    data = json.loads(nc.to_json_bytes())
    n = 0
    for fn in data["functions"]:
        for blk in fn["blocks"]:
            out = []
            for inst in blk["instructions"]:
                si = inst.get("sync_info")
                ow = (si or {}).get("on_wait") or []
                if len(ow) > 1:
                    for w in ow[:-1]:
                        n += 1
                        out.append({
                            "name": f"eswait_{n}",
                            "opcode": "EventSemaphore",
                            "engine": inst["engine"],
                            "ins": [],
                            "outs": [],
                            "sync_info": {"on_wait": [w], "on_update": []},
                        })
                    si["on_wait"] = [ow[-1]]
                out.append(inst)
            blk["instructions"] = out
    nc.m = mybir.module_from_json_bytes(json.dumps(data).encode())
    return nc


def prep_host(inputs: dict):
    x = np.ascontiguousarray(np.asarray(inputs["x"], dtype=np.float32))
    y = np.ascontiguousarray(np.asarray(inputs["y"], dtype=np.float32))
    W1 = np.asarray(inputs["W1"], dtype=np.float32)
    W2 = np.asarray(inputs["W2"], dtype=np.float32)
    g1 = np.asarray(inputs["g1"], dtype=np.float32)
    g2 = np.asarray(inputs["g2"], dtype=np.float32)
    b1 = np.asarray(inputs["b1"], dtype=np.float32)
    b2 = np.asarray(inputs["b2"], dtype=np.float32)
    be1 = np.asarray(inputs["be1"], dtype=np.float32)
    be2 = np.asarray(inputs["be2"], dtype=np.float32)
    W3 = np.asarray(inputs["W3"], dtype=np.float32)
    b3 = np.asarray(inputs["b3"], dtype=np.float32)
    W4 = np.asarray(inputs["W4"], dtype=np.float32)
    b4 = np.asarray(inputs["b4"], dtype=np.float32)
    W5 = np.asarray(inputs["W5"], dtype=np.float32)
    b5 = np.asarray(inputs["b5"], dtype=np.float32)

    W1p = W1 * g1[None, :]
    W2p = W2 * g2[None, :]
    b1p = b1 * g1 + be1
    b2p = b2 * g2 + be2
    W5a, W5b = W5[:C, 0], W5[C:, 0]
    u3 = (W3 @ W5a).astype(np.float32)
    u4 = (W4 @ W5b).astype(np.float32)
    z = (u4[:2 * N] + u4[2 * N:]).astype(np.float32)
    c0 = float(b3 @ W5a + b4 @ W5b + b5[0])

    def pack_w(w):
        # [C, C] -> [DT, 128, KP, 2, 128]: [m-block d, part p, kpair, j, m]
        # = w[(2*kpair+j)*128+p, d*128+m]
        v = w.astype(FP8_NP).reshape(KP, 2, 128, DT, 128)
        return np.ascontiguousarray(v.transpose(3, 2, 0, 1, 4))

    def pack_gemm(a):
        # [B,N,C] -> [M, NG, 128, KP, 2, 400]: token blocks [a(196)|b(196)|pad]
        at = a.transpose(0, 2, 1).reshape(NCORES, S, KP, 2, 128, N)
        pair = at.reshape(NCORES, NG, G, KP, 2, 128, N)
        gg = np.concatenate([pair[:, :, 0], pair[:, :, 1]], axis=-1)
        gg = gg.transpose(0, 1, 4, 2, 3, 5)  # [M,NG,128,KP,2,392]
        out = np.zeros((NCORES, NG, 128, KP, 2, W2TP), dtype=FP8_NP)
        out[..., :W2T] = gg.astype(FP8_NP)
        return out

    def pack_fin(a):
        at = a.transpose(0, 2, 1).reshape(NCORES, S, DT, 128, N)
        pair = at.reshape(NCORES, NG, G, DT, 128, N)
        gg = np.concatenate([pair[:, :, 0], pair[:, :, 1]], axis=-1)
        return np.ascontiguousarray(
            gg.transpose(0, 1, 3, 2, 4).reshape(NCORES, NG, 128, DT * W2T)
            .astype(F16_NP))

    XM, YM = pack_gemm(x), pack_gemm(y)
    XF, YF = pack_fin(x), pack_fin(y)
    zb = np.ascontiguousarray(np.broadcast_to(z[None, :], (128, W2T)).astype(F16_NP))
    u3t = np.ascontiguousarray(u3.reshape(DT, 128).T)
    b1t = np.ascontiguousarray(b1p.reshape(DT, 128).T)
    b2t = np.ascontiguousarray(b2p.reshape(DT, 128).T)

    in_maps = []
    for cidx in range(NCORES):
        in_maps.append({
            "xm": XM[cidx], "ym": YM[cidx], "xf": XF[cidx], "yf": YF[cidx],
            "w1": pack_w(W1p), "w2": pack_w(W2p),
            "zb": zb, "u3": u3t, "b1": b1t, "b2": b2t,
        })
    return in_maps, c0, x, y


def unpack_out(results) -> np.ndarray:
    outs = []
    for cidx in range(NCORES):
        o = np.asarray(results[cidx]["out"]).astype(np.float32)  # [S, 128, DT*N]
        o = o.reshape(S, 128, DT, N).transpose(0, 2, 1, 3).reshape(S, C, N)
        outs.append(o.transpose(0, 2, 1))     # [S, N, C]
    return np.ascontiguousarray(np.concatenate(outs, axis=0))


def kernel(**inputs) -> np.ndarray:
    in_maps, c0, _, _ = prep_host(inputs)
    nc = build_bass(c0)
    res = run_bass_kernel_spmd(nc, in_maps, list(range(NCORES)))
    return unpack_out(res.results)
